# revision 10
# baseline (speedup 1.0000x reference)
"""Trainium2 Bass kernel for nn_MultiHeadAttention_910533067646.

Self-contained: builds the Bass module, shards the full inputs across the
8 NeuronCores (data-parallel over batch x tensor-parallel over heads), runs
via PJRT, and reassembles the full output.

The reference module applies one shared projection p = x @ Wv.T + bv for
q=k=v, per-head softmax(p ph.T/8) @ ph, then a head-major (bugged) reshape
and output projection. The bugged reshape maps each head's attention output
to a disjoint 128-row block of the final output, so no cross-device
reduction is needed: device (b, hg) computes output rows
[1024*hg, 1024*hg+1024) of batch b.

v2 engine split: the softmax exp over S x S scores is the elementwise
bottleneck, so it is spread across three engines. Scores are symmetric
(q=k), so for each 128-row q-block only one of the two 1024-col halves
contains the diagonal block. Diagonal-half tiles (plus one fixed off-diag
row per half) run exact exp+row-accumulate on the Activation engine; the
remaining off-diagonal tiles are computed by a Schraudolph bit-trick exp on
the Vector engine (y = int32(s*a+b) bitcast as fp32) and their softmax-
denominator contributions come from GpSimd partition-axis column sums
(valid by symmetry). The per-column denominators are assembled k-indexed by
a ones-matmul broadcast, reciprocated on DVE, and applied to the
attention-value product.
"""
import math
import numpy as np

from collections import deque
from contextlib import ExitStack

import concourse.bass as bass
import concourse.mybir as mybir
import concourse.tile as tile
from concourse.masks import make_identity

FP = mybir.dt.float32
FP16 = mybir.dt.float16
BF16 = mybir.dt.bfloat16
I16 = mybir.dt.int16
Exp = mybir.ActivationFunctionType.Exp
ADD = mybir.AluOpType.add
MULT = mybir.AluOpType.mult

BIAS = -15.0                                  # static logit shift (softmax-invariant)
SCHR_SCALE = 2.0 ** 7 / math.log(2.0)         # 184.664965 (bf16 target)
SCHR_A = SCHR_SCALE / 8.0                     # exp arg is scores/8
SCHR_B = 127.0 * 2.0 ** 7 - 7.35 + BIAS * SCHR_SCALE


def _build_mha_nc(S=2048, D=1024, HL=8, dk=64, phases="ABCNF",
                  loop_bcnf=1, dbg=False):
    EL = HL * dk            # local width of the value projection
    KK = D // 128           # contraction k-tiles
    NG = HL // 2            # head pairs
    NB = S // 128           # 128-row blocks of the sequence
    NBH = NB // 2           # blocks per sq-half
    SQH = S // 2            # sq-half width
    TT = D // dk            # total heads (= reshape block count)
    W = min(512, SQH)       # N-slice width for panels
    NSL = SQH // W
    WS = min(512, S)        # N-slice for pT phase
    NSS = S // WS
    WD = min(512, D)        # N-slice over D (output projection)
    NSD = D // WD
    ACT_ROW = NBH - 1       # off-diag row (i % NBH) handled by Act engine
    POOLW = ACT_ROW * 128   # gpsimd col-reduce width (excludes ACT_ROW block)
    assert EL <= 512 and SQH == D and S == 128 * TT and TT % 2 == 0

    nc = bass.Bass("TRN2")
    xT_d = nc.dram_tensor("xT", [D, S], FP16, kind="ExternalInput")
    wvT_d = nc.dram_tensor("wvT", [D, EL], FP16, kind="ExternalInput")
    woT_d = nc.dram_tensor("woT", [D, D], FP16, kind="ExternalInput")
    bv_d = nc.dram_tensor("bv", [1, EL], FP16, kind="ExternalInput")
    bo_d = nc.dram_tensor("bo", [1, D], FP16, kind="ExternalInput")
    sel_d = nc.dram_tensor("sel", [18, 128], FP, kind="ExternalInput")
    out_d = nc.dram_tensor("out", [128 * HL, D], FP, kind="ExternalOutput")

    with ExitStack() as stk:
        tc = stk.enter_context(tile.TileContext(nc))
        const = stk.enter_context(tc.tile_pool(name="const", bufs=1))
        ppool = stk.enter_context(tc.tile_pool(name="ppool", bufs=1))
        epool = stk.enter_context(tc.tile_pool(name="epool", bufs=8))
        rpool = stk.enter_context(tc.tile_pool(name="rpool", bufs=4))
        ps_m = stk.enter_context(tc.tile_pool(name="ps_m", bufs=3, space="PSUM"))

        bv_sb = const.tile([1, EL], FP16, name="bv_sb")
        bo_sb = const.tile([1, D], FP16, name="bo_sb")
        ones16 = const.tile([1, 512], FP16, name="ones16")
        sel_sb = const.tile([18, 128], FP, name="sel_sb")
        ident = const.tile([128, 128], FP, name="ident")
        identB = const.tile([128, 128], BF16, name="identB")
        bias_sb = const.tile([128, 1], FP, name="bias_sb")
        nc.sync.dma_start(bv_sb[:], bv_d[:])
        nc.sync.dma_start(bo_sb[:], bo_d[:])
        nc.gpsimd.memset(ones16[:], 1.0)
        nc.gpsimd.memset(bias_sb[:], BIAS)
        nc.sync.dma_start(sel_sb[:], sel_d[:])
        make_identity(nc, ident[:])
        make_identity(nc, identB[:])

        pT_sb = ppool.tile([128, NG, S], BF16, name="pT_sb")
        p_sb = ppool.tile([128, NB, EL], BF16, name="p_sb")

        xt_ctx = tc.tile_pool(name="xtpool", bufs=1)
        xtpool = xt_ctx.__enter__()
        wvT_sb = xtpool.tile([128, KK, EL], FP16, name="wvT_sb")
        xT_sb = xtpool.tile([128, KK, S], FP16, name="xT_sb")
        nc.sync.dma_start(wvT_sb[:],
                          wvT_d[:].rearrange("(kk p) e -> p kk e", p=128))
        for kk in range(KK):
            nc.sync.dma_start(xT_sb[:, kk, :], xT_d[128 * kk:128 * (kk + 1), :])

        # ---- projection work units (phase A), emitted interleaved ----
        def emit_pT0(ns):
            ps = ps_m.tile([128, WS], FP, name="ps_pt", tag="scores")
            for kk in range(KK):
                nc.tensor.matmul(ps[:], wvT_sb[:, kk, 0:128],
                                 xT_sb[:, kk, WS * ns:WS * (ns + 1)],
                                 start=(kk == 0), stop=False)
            nc.tensor.matmul(ps[:], bv_sb[0:1, 0:128],
                             ones16[0:1, 0:WS], start=False, stop=True)
            nc.vector.tensor_copy(pT_sb[:, 0, WS * ns:WS * (ns + 1)], ps[:])

        def emit_p(j):
            ps = ps_m.tile([128, EL], FP, name="ps_p", tag="scores")
            for kk in range(KK):
                nc.tensor.matmul(ps[:], xT_sb[:, kk, 128 * j:128 * (j + 1)],
                                 wvT_sb[:, kk, :], start=(kk == 0), stop=False)
            nc.tensor.matmul(ps[:], ones16[0:1, 0:128], bv_sb[0:1, :],
                             start=False, stop=True)
            nc.vector.tensor_copy(p_sb[:, j, :], ps[:])

        def emit_T(g, j):
            ps = ps_m.tile([128, 128], BF16, name="ps_t0", tag="scores")
            nc.tensor.transpose(ps[:], p_sb[:, j, 128 * g:128 * (g + 1)],
                                identB[:])
            nc.vector.tensor_copy(pT_sb[:, g, 128 * j:128 * (j + 1)], ps[:])

        proj_q = deque()
        for j in range(NB):
            proj_q.append(("p", j))
        for g in range(1, NG):
            for j in range(NB):
                proj_q.append(("T", g, j))
        p_left = [NB]

        post_pools = {}

        def ensure_post_pools():
            # opened once the p-projection is done: reuses xT address space
            if post_pools:
                return
            xt_ctx.__exit__(None, None, None)
            post_pools["w"] = stk.enter_context(tc.tile_pool(name="wpool", bufs=1))
            post_pools["n"] = stk.enter_context(tc.tile_pool(name="npool", bufs=2))
            post_pools["b"] = stk.enter_context(tc.tile_pool(name="bpool", bufs=2))
            post_pools["f"] = stk.enter_context(tc.tile_pool(name="fpool", bufs=2))
            woT_dup = post_pools["w"].tile([128, TT, D], FP16, name="woT_dup")
            src = woT_d[:].rearrange("(t p) e -> p t e", p=dk)
            nc.sync.dma_start(woT_dup[0:dk, :, :], src)
            nc.sync.dma_start(woT_dup[dk:2 * dk, :, :], src)
            post_pools["woT"] = woT_dup

        def emit_proj(n):
            while n > 0 and proj_q:
                u = proj_q.popleft()
                if u[0] == "p":
                    emit_p(u[1])
                    p_left[0] -= 1
                    if p_left[0] == 0:
                        ensure_post_pools()
                else:
                    emit_T(u[1], u[2])
                n -= 1

        for ns in range(NSS):
            emit_pT0(ns)

        if "B" not in phases:
            emit_proj(len(proj_q))
            ensure_post_pools()

        loop_cm = None
        if loop_bcnf > 1:
            emit_proj(len(proj_q))
            ensure_post_pools()
            loop_cm = tc.For_i(0, loop_bcnf, 1)
            loop_cm.__enter__()
        pending_nf = [None]
        for g in range(NG if "B" in phases else 0):
            sums = epool.tile([128, 2, NB, 2], FP, name="sums", tag="sums", bufs=2)
            nc.vector.memset(sums[:], 0.0)
            outT_sb_box = [None]
            cpart = [None, None]
            rows_hh = [None, None]

            W16 = min(512, SQH)
            NS16 = SQH // W16

            def emit_C_one(h, i, ns, a2, E):
                al = 2 * g + a2
                if cpart[h] is None:
                    cpart[h] = ps_m.tile([128, SQH], FP, name="cp",
                                         tag="cpart", bufs=1)
                nc.tensor.matmul(
                    cpart[h][64 * a2:64 * (a2 + 1), W16 * ns:W16 * (ns + 1)],
                    p_sb[:, i, dk * al:dk * (al + 1)],
                    E[:, W16 * ns:W16 * (ns + 1)],
                    tile_position=(0, 64 * a2),
                    start=(i == 0), stop=(i == NB - 1),
                    skip_group_check=True)

            def drain_C(h):
                if outT_sb_box[0] is None:
                    outT_sb_box[0] = post_pools["n"].tile(
                        [128, 2, SQH], FP, name="outT_sb", tag="outT_sb", bufs=2)
                nc.vector.tensor_copy(outT_sb_box[0][:, h, :], cpart[h][:])
                cpart[h] = None

            for h in range(2):
                rows_h = rpool.tile([18, SQH], FP, name="rows", tag="rows", bufs=4)
                rows_hh[h] = rows_h
                nc.gpsimd.memset(rows_h[:, POOLW:SQH], 0.0)
                prev = None
                for i in range(NB):
                    emit_proj(2)
                    if h == 0 and i == 2 and pending_nf[0] is not None:
                        pending_nf[0]()
                        pending_nf[0] = None
                    cur = []
                    for a2 in range(2):
                        lo, hi = 64 * a2, 64 * (a2 + 1)
                        # C matmuls of this head for step i-1 first: they
                        # depend only on this head's E(i-1), so they fill
                        # the PE gap while the other head's exp still runs
                        if prev is not None and "C" in phases:
                            for k in range(NS16):
                                ns = (k + a2) % NS16
                                emit_C_one(h, i - 1, ns, a2, prev[a2])
                        sc = ps_m.tile([128, SQH], FP, name="sc", tag="scores")
                        for ns in range(NSL):
                            nc.tensor.matmul(
                                sc[:, W * ns:W * (ns + 1)],
                                pT_sb[lo:hi, g, 128 * i:128 * (i + 1)],
                                pT_sb[lo:hi, g,
                                      SQH * h + W * ns:SQH * h + W * (ns + 1)],
                                tile_position=(64 * a2, 0))
                        E = epool.tile([128, SQH], BF16, name="E", tag="E", bufs=8)
                        diag = (i // NBH) == h
                        if diag or (i % NBH) == ACT_ROW:
                            nc.scalar.activation(
                                E[:], sc[:], Exp,
                                scale=0.125, bias=bias_sb[:, 0:1],
                                accum_out=sums[:, a2, i, h:h + 1])
                        else:
                            nc.vector.tensor_scalar(
                                E[:].bitcast(I16), sc[:],
                                SCHR_A, SCHR_B, MULT, ADD)
                        if not diag:
                            # gpsimd reduce output must land on partition 0;
                            # DMA scatters it to its row of the stack
                            crow = rpool.tile([1, SQH], FP, name="crow",
                                              tag="crow", bufs=4)
                            nc.gpsimd.reduce_sum(crow[0:1, 0:POOLW],
                                                 E[:, 0:POOLW],
                                                 axis=mybir.AxisListType.C)
                            slot = 9 * a2 + (i % NBH)
                            nc.sync.dma_start(
                                rows_h[slot:slot + 1, 0:POOLW],
                                crow[0:1, 0:POOLW])
                        cur.append(E)
                    prev = cur
                if "C" in phases:
                    for k in range(NS16):
                        for a2 in range(2):
                            emit_C_one(h, NB - 1, (k + a2) % NS16, a2, prev[a2])
                    drain_C(h)

            emit_proj(len(proj_q))  # flush any phase-A leftovers
            ensure_post_pools()
            woT_dup = post_pools["woT"]
            if "N" not in phases:
                continue

            # ---- normalization + output projection ----

            def make_nf(g=g, sums=sums, outT_sb_box=outT_sb_box,
                        rows_hh=tuple(rows_hh)):
                def nf():
                    # diag-half row sums, transposed to k-indexing
                    totT = rpool.tile([NB, 2, 128], FP, name="totT",
                                      tag="totT", bufs=2)
                    tot = epool.tile([128, 2, NB], FP, name="tot", tag="tot",
                                     bufs=2)
                    for a2 in range(2):
                        nc.vector.tensor_tensor(tot[:, a2, :], sums[:, a2, :, 0],
                                                sums[:, a2, :, 1], ADD)
                        ps_t = ps_m.tile([NB, 128], FP, name="ps_tt", tag="scores")
                        nc.tensor.transpose(ps_t[:], tot[:, a2, :], ident[:])
                        nc.vector.tensor_copy(totT[:, a2, :], ps_t[:])
                    norm_g = post_pools["n"].tile([128, S], FP16, name="norm_g",
                                                  tag="nr")
                    for h in range(2):
                        rows_h = rows_hh[h]
                        for a2 in range(2):
                            nc.sync.dma_start(
                                rows_h[9 * a2 + 8:9 * a2 + 9, :],
                                totT[NBH * h:NBH * (h + 1), a2, :])
                        # bc[p, n] = D[n] for the a2-half p belongs to
                        bc_ps = ps_m.tile([128, SQH], FP, name="bc_ps",
                                          tag="scores")
                        for ns in range(NSL):
                            nc.tensor.matmul(bc_ps[:, W * ns:W * (ns + 1)],
                                             sel_sb[:],
                                             rows_h[:, W * ns:W * (ns + 1)])
                        bc = post_pools["b"].tile([128, SQH], FP, name="bc",
                                                  tag="bc")
                        nc.vector.reciprocal(bc[:], bc_ps[:])
                        nc.vector.tensor_tensor(norm_g[:, SQH * h:SQH * (h + 1)],
                                                outT_sb_box[0][:, h, :], bc[:],
                                                MULT)

                    if "F" not in phases:
                        return
                    # ---- output projection (4-quadrant: a2 on rows, h on cols) ----
                    fps = [ps_m.tile([128, D], FP, name="fp_a", tag="scores"),
                           ps_m.tile([128, D], FP, name="fp_b", tag="cpart",
                                     bufs=1)]
                    for a2 in range(2):
                        for ns in range(NSD):
                            nc.tensor.matmul(fps[a2][:, WD * ns:WD * (ns + 1)],
                                             ones16[0:1, 0:128],
                                             bo_sb[0:1, WD * ns:WD * (ns + 1)],
                                             start=True, stop=False,
                                             skip_group_check=True)
                    for ns in range(NSD):
                        for t in range(TT):
                            for a2 in range(2):
                                lo = 64 * a2
                                nc.tensor.matmul(
                                    fps[a2][:, WD * ns:WD * (ns + 1)],
                                    norm_g[lo:lo + 64, t::TT],
                                    woT_dup[lo:lo + 64, t, WD * ns:WD * (ns + 1)],
                                    tile_position=(lo, 0),
                                    start=False, stop=(t == TT - 1),
                                    skip_group_check=True)
                    for a2 in range(2):
                        fsb = post_pools["f"].tile([128, D], FP, name="fsb",
                                                   tag="fsb")
                        nc.vector.tensor_copy(fsb[:], fps[a2][:])
                        al = 2 * g + a2
                        nc.sync.dma_start(out_d[128 * al:128 * (al + 1), :],
                                          fsb[:])

                return nf

            pending_nf[0] = make_nf()

        if pending_nf[0] is not None:
            pending_nf[0]()
            pending_nf[0] = None
        if loop_cm is not None:
            loop_cm.__exit__(None, None, None)

    return nc


def _split_excess_waits(nc, max_waits=1):
    """This toolchain's walrus accepts only one sync-wait per instruction;
    hoist extra waits onto NoOps inserted just before."""
    fn = nc.m.functions[0]
    n_new = 0
    for blk in fn.blocks:
        new_insts = []
        for inst in blk.instructions:
            si = getattr(inst, 'sync_info', None)
            if si is not None and si.on_wait is not None \
                    and len(si.on_wait) > max_waits:
                waits = list(si.on_wait)
                while len(waits) > max_waits:
                    chunk, waits = waits[:max_waits], waits[max_waits:]
                    n_new += 1
                    new_insts.append(mybir.InstNoOp(
                        name=f"I-waitsplit-{n_new}", engine=inst.engine,
                        ins=[], outs=[],
                        sync_info=mybir.SyncInfo(on_wait=chunk, on_update=[]),
                        bass_nofuse=True))
                si.on_wait = waits
            new_insts.append(inst)
        blk.instructions = new_insts
    return n_new


class _PjrtRunner:
    def __init__(self, nc, n_cores):
        import jax
        from jax.sharding import Mesh, PartitionSpec
        from jax.experimental.shard_map import shard_map
        from concourse.bass2jax import (_bass_exec_p, partition_id_tensor,
                                        install_neuronx_cc_hook)
        install_neuronx_cc_hook()
        self.jax = jax
        self.n_cores = n_cores
        pname = nc.partition_id_tensor.name if nc.partition_id_tensor else None
        in_names, out_names, out_avals, zero_outs = [], [], [], []
        for alloc in nc.m.functions[0].allocations:
            if not isinstance(alloc, mybir.MemoryLocationSet):
                continue
            name = alloc.memorylocations[0].name
            if alloc.kind == "ExternalInput":
                if name != pname:
                    in_names.append(name)
            elif alloc.kind == "ExternalOutput":
                shape = tuple(alloc.tensor_shape)
                dtype = mybir.dt.np(alloc.dtype)
                out_names.append(name)
                out_avals.append(jax.core.ShapedArray(shape, dtype))
                zero_outs.append(np.zeros(shape, dtype))
        self.in_names, self.out_names = in_names, out_names
        self.out_avals, self.zero_outs = out_avals, zero_outs
        n_params, n_outs = len(in_names), len(out_avals)
        self.n_params = n_params
        all_in = in_names + out_names + ([pname] if pname else [])

        def _body(*args):
            operands = list(args)
            if pname is not None:
                operands.append(partition_id_tensor())
            return tuple(_bass_exec_p.bind(
                *operands, out_avals=tuple(out_avals), in_names=tuple(all_in),
                out_names=tuple(out_names), lowering_input_output_aliases=(),
                sim_require_finite=True, sim_require_nnan=True, nc=nc))

        devices = jax.devices()[:n_cores]
        self.mesh = Mesh(np.asarray(devices), ("core",))
        in_specs = (PartitionSpec("core"),) * (n_params + n_outs)
        out_specs = (PartitionSpec("core"),) * n_outs
        self.fn = jax.jit(
            shard_map(_body, mesh=self.mesh, in_specs=in_specs,
                      out_specs=out_specs, check_rep=False), keep_unused=True)
        self.PartitionSpec = PartitionSpec

    def run(self, in_maps):
        jax = self.jax
        per_core = [[np.asarray(m[n]) for n in self.in_names] for m in in_maps]
        concat_in = [np.concatenate([per_core[c][i] for c in range(self.n_cores)],
                                    axis=0) for i in range(self.n_params)]
        concat_zeros = [np.zeros((self.n_cores * z.shape[0], *z.shape[1:]),
                                 z.dtype) for z in self.zero_outs]
        sharding = jax.sharding.NamedSharding(self.mesh, self.PartitionSpec("core"))
        dev_in = [jax.device_put(a, sharding) for a in concat_in + concat_zeros]
        outs = self.fn(*dev_in)
        jax.block_until_ready(outs)
        return [
            {n: np.asarray(outs[i]).reshape(self.n_cores,
                                            *self.out_avals[i].shape)[c]
             for i, n in enumerate(self.out_names)}
            for c in range(self.n_cores)
        ]


_CACHE = {}

B_, S_, D_, H_, DK_ = 4, 2048, 1024, 16, 64
HL_ = H_ // 2          # heads per device
EL_ = HL_ * DK_        # value-projection width per device

# bc broadcast selector: rows 0-8 are head a2=0 (partitions 0-63),
# rows 9-17 head a2=1 (partitions 64-127)
_SEL = np.zeros((18, 128), np.float32)
_SEL[0:9, 0:64] = 1.0
_SEL[9:18, 64:128] = 1.0


def _make_in_maps(x, Wv, bv, Wo, bo):
    woT = np.ascontiguousarray(Wo.T).astype(np.float16)
    bo16 = bo.reshape(1, -1).astype(np.float16)
    maps = []
    for dev in range(8):
        b, hg = dev // 2, dev % 2
        maps.append({
            "xT": np.ascontiguousarray(x[b].T).astype(np.float16),
            "wvT": np.ascontiguousarray(
                Wv[EL_ * hg:EL_ * (hg + 1), :].T).astype(np.float16),
            "woT": woT,
            "bv": bv[EL_ * hg:EL_ * (hg + 1)].reshape(1, -1).astype(np.float16),
            "bo": bo16,
            "sel": _SEL,
        })
    return maps


def kernel(x, Wv, bv, Wo, bo):
    x, Wv, bv = np.asarray(x), np.asarray(Wv), np.asarray(bv)
    Wo, bo = np.asarray(Wo), np.asarray(bo)
    if "r" not in _CACHE:
        nc = _build_mha_nc(S=S_, D=D_, HL=HL_, dk=DK_)
        _split_excess_waits(nc)
        _CACHE["r"] = _PjrtRunner(nc, 8)
    r = _CACHE["r"]
    res = r.run(_make_in_maps(x, Wv, bv, Wo, bo))
    out = np.zeros((B_, S_, D_), np.float32)
    for dev in range(8):
        b, hg = dev // 2, dev % 2
        out[b, 1024 * hg:1024 * (hg + 1), :] = res[dev]["out"]
    return out


# revision 16
# speedup vs baseline: 19.0925x; 19.0925x over previous
"""Trainium2 Bass kernel for nn_MultiHeadAttention_910533067646.

Self-contained: builds the Bass module, shards the full inputs across the
8 NeuronCores (data-parallel over batch x tensor-parallel over heads), runs
via PJRT, and reassembles the full output.

The reference module applies one shared projection p = x @ Wv.T + bv for
q=k=v, per-head softmax(p ph.T/8) @ ph, then a head-major (bugged) reshape
and output projection. The bugged reshape maps each head's attention output
to a disjoint 128-row block of the final output, so no cross-device
reduction is needed: device (b, hg) computes output rows
[1024*hg, 1024*hg+1024) of batch b.

v2 engine split: the softmax exp over S x S scores is the elementwise
bottleneck, so it is spread across three engines. Scores are symmetric
(q=k), so for each 128-row q-block only one of the two 1024-col halves
contains the diagonal block. Diagonal-half tiles (plus one fixed off-diag
row per half) run exact exp+row-accumulate on the Activation engine; the
remaining off-diagonal tiles are computed by a Schraudolph bit-trick exp on
the Vector engine (y = int32(s*a+b) bitcast as fp32) and their softmax-
denominator contributions come from GpSimd partition-axis column sums
(valid by symmetry). The per-column denominators are assembled k-indexed by
a ones-matmul broadcast, reciprocated on DVE, and applied to the
attention-value product.
"""
import math
import numpy as np

from collections import deque
from contextlib import ExitStack

import concourse.bass as bass
import concourse.mybir as mybir
import concourse.tile as tile
from concourse.masks import make_identity

FP = mybir.dt.float32
FP16 = mybir.dt.float16
BF16 = mybir.dt.bfloat16
I16 = mybir.dt.int16
Exp = mybir.ActivationFunctionType.Exp
ADD = mybir.AluOpType.add
MULT = mybir.AluOpType.mult

BIAS = -15.0                                  # static logit shift (softmax-invariant)
SCHR_SCALE = 2.0 ** 7 / math.log(2.0)         # 184.664965 (bf16 target)
SCHR_A = SCHR_SCALE / 8.0                     # exp arg is scores/8
SCHR_B = 127.0 * 2.0 ** 7 - 7.35 + BIAS * SCHR_SCALE


def _build_mha_nc(S=2048, D=1024, HL=8, dk=64, phases="ABCNF",
                  loop_bcnf=1, dbg=False):
    EL = HL * dk            # local width of the value projection
    KK = D // 128           # contraction k-tiles
    NG = HL // 2            # head pairs
    NB = S // 128           # 128-row blocks of the sequence
    NBH = NB // 2           # blocks per sq-half
    SQH = S // 2            # sq-half width
    TT = D // dk            # total heads (= reshape block count)
    W = min(512, SQH)       # N-slice width for panels
    NSL = SQH // W
    WS = min(512, S)        # N-slice for pT phase
    NSS = S // WS
    WD = min(512, D)        # N-slice over D (output projection)
    NSD = D // WD
    ACT_ROW = NBH - 1       # off-diag row (i % NBH) handled by Act engine
    POOLW = ACT_ROW * 128   # gpsimd col-reduce width (excludes ACT_ROW block)
    assert EL <= 512 and SQH == D and S == 128 * TT and TT % 2 == 0

    nc = bass.Bass("TRN2")
    xT_d = nc.dram_tensor("xT", [D, S], FP16, kind="ExternalInput")
    wvT_d = nc.dram_tensor("wvT", [D, EL], FP16, kind="ExternalInput")
    woT_d = nc.dram_tensor("woT", [D, D], FP16, kind="ExternalInput")
    bv_d = nc.dram_tensor("bv", [1, EL], FP16, kind="ExternalInput")
    bo_d = nc.dram_tensor("bo", [1, D], FP16, kind="ExternalInput")
    sel_d = nc.dram_tensor("sel", [33, 128], FP, kind="ExternalInput")
    out_d = nc.dram_tensor("out", [128 * HL, D], FP, kind="ExternalOutput")

    with ExitStack() as stk:
        tc = stk.enter_context(tile.TileContext(nc))
        const = stk.enter_context(tc.tile_pool(name="const", bufs=1))
        ppool = stk.enter_context(tc.tile_pool(name="ppool", bufs=1))
        epool = stk.enter_context(tc.tile_pool(name="epool", bufs=8))
        rpool = stk.enter_context(tc.tile_pool(name="rpool", bufs=4))
        ps_m = stk.enter_context(tc.tile_pool(name="ps_m", bufs=2, space="PSUM"))

        bv_sb = const.tile([1, EL], FP16, name="bv_sb")
        bo_sb = const.tile([1, D], FP16, name="bo_sb")
        ones16 = const.tile([1, 512], FP16, name="ones16")
        sel_sb = const.tile([33, 128], FP, name="sel_sb")
        ident = const.tile([128, 128], FP, name="ident")
        identB = const.tile([128, 128], BF16, name="identB")
        bias_sb = const.tile([128, 1], FP, name="bias_sb")
        nc.sync.dma_start(bv_sb[:], bv_d[:])
        nc.sync.dma_start(bo_sb[:], bo_d[:])
        ones_bf = const.tile([128, 1], BF16, name="ones_bf")
        nc.gpsimd.memset(ones16[:], 1.0)
        nc.gpsimd.memset(ones_bf[:], 1.0)
        nc.gpsimd.memset(bias_sb[:], BIAS)
        nc.sync.dma_start(sel_sb[:], sel_d[:])
        make_identity(nc, ident[:])
        make_identity(nc, identB[:])

        pT_sb = ppool.tile([128, NG, S], BF16, name="pT_sb")
        p_sb = ppool.tile([128, NB, EL], BF16, name="p_sb")

        xt_ctx = tc.tile_pool(name="xtpool", bufs=1)
        xtpool = xt_ctx.__enter__()
        wvT_sb = xtpool.tile([128, KK, EL], FP16, name="wvT_sb")
        xT_sb = xtpool.tile([128, KK, S], FP16, name="xT_sb")
        nc.sync.dma_start(wvT_sb[:],
                          wvT_d[:].rearrange("(kk p) e -> p kk e", p=128))
        for kk in range(KK):
            nc.sync.dma_start(xT_sb[:, kk, :], xT_d[128 * kk:128 * (kk + 1), :])

        # ---- projection work units (phase A), emitted interleaved ----
        def emit_pT0(ns):
            ps = ps_m.tile([128, WS], FP, name="ps_pt", tag="scores")
            for kk in range(KK):
                nc.tensor.matmul(ps[:], wvT_sb[:, kk, 0:128],
                                 xT_sb[:, kk, WS * ns:WS * (ns + 1)],
                                 start=(kk == 0), stop=False)
            nc.tensor.matmul(ps[:], bv_sb[0:1, 0:128],
                             ones16[0:1, 0:WS], start=False, stop=True)
            nc.vector.tensor_copy(pT_sb[:, 0, WS * ns:WS * (ns + 1)], ps[:])

        def emit_p(j):
            ps = ps_m.tile([128, EL], FP, name="ps_p", tag="scores")
            for kk in range(KK):
                nc.tensor.matmul(ps[:], xT_sb[:, kk, 128 * j:128 * (j + 1)],
                                 wvT_sb[:, kk, :], start=(kk == 0), stop=False)
            nc.tensor.matmul(ps[:], ones16[0:1, 0:128], bv_sb[0:1, :],
                             start=False, stop=True)
            nc.vector.tensor_copy(p_sb[:, j, :], ps[:])

        def emit_T(g, j):
            ps = ps_m.tile([128, 128], BF16, name="ps_t0", tag="scores")
            nc.tensor.transpose(ps[:], p_sb[:, j, 128 * g:128 * (g + 1)],
                                identB[:])
            nc.vector.tensor_copy(pT_sb[:, g, 128 * j:128 * (j + 1)], ps[:])

        proj_q = deque()
        for j in range(NB):
            proj_q.append(("p", j))
        for g in range(1, NG):
            for j in range(NB):
                proj_q.append(("T", g, j))
        p_left = [NB]

        post_pools = {}

        def ensure_post_pools():
            # opened once the p-projection is done: reuses xT address space
            if post_pools:
                return
            xt_ctx.__exit__(None, None, None)
            post_pools["w"] = stk.enter_context(tc.tile_pool(name="wpool", bufs=1))
            post_pools["n"] = stk.enter_context(tc.tile_pool(name="npool", bufs=2))
            post_pools["b"] = stk.enter_context(tc.tile_pool(name="bpool", bufs=2))
            post_pools["f"] = stk.enter_context(tc.tile_pool(name="fpool", bufs=2))
            woT_dup = post_pools["w"].tile([128, TT, D], FP16, name="woT_dup")
            src = woT_d[:].rearrange("(t p) e -> p t e", p=dk)
            nc.sync.dma_start(woT_dup[0:dk, :, :], src)
            nc.sync.dma_start(woT_dup[dk:2 * dk, :, :], src)
            post_pools["woT"] = woT_dup

        def emit_proj(n):
            while n > 0 and proj_q:
                u = proj_q.popleft()
                if u[0] == "p":
                    emit_p(u[1])
                    p_left[0] -= 1
                    if p_left[0] == 0:
                        ensure_post_pools()
                else:
                    emit_T(u[1], u[2])
                n -= 1

        for ns in range(NSS):
            emit_pT0(ns)

        if "B" not in phases:
            emit_proj(len(proj_q))
            ensure_post_pools()

        loop_cm = None
        if loop_bcnf > 1:
            emit_proj(len(proj_q))
            ensure_post_pools()
            loop_cm = tc.For_i(0, loop_bcnf, 1)
            loop_cm.__enter__()
        pending_nf = [None]
        for g in range(NG if "B" in phases else 0):
            sums = epool.tile([128, 2, NB, 2], FP, name="sums", tag="sums", bufs=2)
            nc.vector.memset(sums[:], 0.0)
            outT_sb_box = [None]
            cpart = [None, None]
            rows_hh = [None, None]

            W16 = min(512, SQH)
            NS16 = SQH // W16

            csum_box = [None]

            def emit_csum(h, i, a2, E):
                # colsum of off-diag tile accumulated over the 8 tiles of
                # this (h, a2) group (PE ones-matmul; excludes ACT_ROW block)
                i0 = NBH * (1 - h)
                if csum_box[0] is None:
                    csum_box[0] = ps_m.tile([33, SQH], FP, name="csum",
                                            tag="csum", bufs=1)
                for ns in range(NSL):
                    wlo, whi = W * ns, min(W * (ns + 1), POOLW)
                    if wlo >= POOLW:
                        break
                    nc.tensor.matmul(csum_box[0][32 * a2:32 * a2 + 1, wlo:whi],
                                     ones_bf[:], E[:, wlo:whi],
                                     start=(i == i0), stop=(i == i0 + NBH - 1),
                                     skip_group_check=True)

            def emit_C_one(h, i, ns, a2, E):
                al = 2 * g + a2
                if cpart[h] is None:
                    cpart[h] = ps_m.tile([128, SQH], FP, name="cp",
                                         tag="cpart", bufs=1)
                nc.tensor.matmul(
                    cpart[h][64 * a2:64 * (a2 + 1), W16 * ns:W16 * (ns + 1)],
                    p_sb[:, i, dk * al:dk * (al + 1)],
                    E[:, W16 * ns:W16 * (ns + 1)],
                    tile_position=(0, 64 * a2),
                    start=(i == 0), stop=(i == NB - 1),
                    skip_group_check=True)

            def drain_C(h):
                if outT_sb_box[0] is None:
                    outT_sb_box[0] = post_pools["n"].tile(
                        [128, 2, SQH], FP, name="outT_sb", tag="outT_sb", bufs=2)
                nc.vector.tensor_copy(outT_sb_box[0][:, h, :], cpart[h][:])
                cpart[h] = None

            for h in range(2):
                rows_h = rpool.tile([33, SQH], FP, name="rows", tag="rows", bufs=4)
                rows_hh[h] = rows_h
                nc.gpsimd.memset(rows_h[:], 0.0)
                prev = None
                for i in range(NB):
                    emit_proj(2)
                    if h == 0 and i == 2 and pending_nf[0] is not None:
                        pending_nf[0]()
                        pending_nf[0] = None
                    cur = []
                    for a2 in range(2):
                        lo, hi = 64 * a2, 64 * (a2 + 1)
                        # C matmuls of this head for step i-1 first: they
                        # depend only on this head's E(i-1), so they fill
                        # the PE gap while the other head's exp still runs
                        if prev is not None and "C" in phases:
                            for k in range(NS16):
                                ns = (k + a2) % NS16
                                emit_C_one(h, i - 1, ns, a2, prev[a2])
                            if ((i - 1) // NBH) != h:
                                emit_csum(h, i - 1, a2, prev[a2])
                        sc = ps_m.tile([128, SQH], FP, name="sc", tag="scores")
                        for ns in range(NSL):
                            nc.tensor.matmul(
                                sc[:, W * ns:W * (ns + 1)],
                                pT_sb[lo:hi, g, 128 * i:128 * (i + 1)],
                                pT_sb[lo:hi, g,
                                      SQH * h + W * ns:SQH * h + W * (ns + 1)],
                                tile_position=(64 * a2, 0))
                        E = epool.tile([128, SQH], BF16, name="E", tag="E", bufs=8)
                        diag = (i // NBH) == h
                        if diag or (i % NBH) == ACT_ROW:
                            nc.scalar.activation(
                                E[:], sc[:], Exp,
                                scale=0.125, bias=bias_sb[:, 0:1],
                                accum_out=sums[:, a2, i, h:h + 1])
                        else:
                            nc.vector.tensor_scalar(
                                E[:].bitcast(I16), sc[:],
                                SCHR_A, SCHR_B, MULT, ADD)
                        cur.append(E)
                    prev = cur
                if "C" in phases:
                    for k in range(NS16):
                        for a2 in range(2):
                            emit_C_one(h, NB - 1, (k + a2) % NS16, a2, prev[a2])
                    for a2 in range(2):
                        if ((NB - 1) // NBH) != h:
                            emit_csum(h, NB - 1, a2, prev[a2])
                    drain_C(h)
                    # csum rows -> rows_h (lane-aligned copies: partitions 0, 32)
                    nc.vector.tensor_copy(rows_h[0:1, 0:POOLW],
                                          csum_box[0][0:1, 0:POOLW])
                    nc.scalar.copy(rows_h[32:33, 0:POOLW],
                                   csum_box[0][32:33, 0:POOLW])
                    csum_box[0] = None

            emit_proj(len(proj_q))  # flush any phase-A leftovers
            ensure_post_pools()
            woT_dup = post_pools["woT"]
            if "N" not in phases:
                continue

            # ---- normalization + output projection ----

            def make_nf(g=g, sums=sums, outT_sb_box=outT_sb_box,
                        rows_hh=tuple(rows_hh)):
                def nf():
                    # diag-half row sums, transposed to k-indexing
                    totT = rpool.tile([NB, 2, 128], FP, name="totT",
                                      tag="totT", bufs=2)
                    tot = epool.tile([128, 2, NB], FP, name="tot", tag="tot",
                                     bufs=2)
                    for a2 in range(2):
                        nc.vector.tensor_tensor(tot[:, a2, :], sums[:, a2, :, 0],
                                                sums[:, a2, :, 1], ADD)
                        ps_t = ps_m.tile([NB, 128], FP, name="ps_tt", tag="scores")
                        nc.tensor.transpose(ps_t[:], tot[:, a2, :], ident[:])
                        nc.vector.tensor_copy(totT[:, a2, :], ps_t[:])
                    norm_g = post_pools["n"].tile([128, S], FP16, name="norm_g",
                                                  tag="nr")
                    for h in range(2):
                        rows_h = rows_hh[h]
                        for a2 in range(2):
                            nc.sync.dma_start(
                                rows_h[a2 + 1:a2 + 2, :],
                                totT[NBH * h:NBH * (h + 1), a2, :])
                        # bc[p, n] = D[n] for the a2-half p belongs to
                        bc_ps = ps_m.tile([128, SQH], FP, name="bc_ps",
                                          tag="scores")
                        for ns in range(NSL):
                            nc.tensor.matmul(bc_ps[:, W * ns:W * (ns + 1)],
                                             sel_sb[:],
                                             rows_h[:, W * ns:W * (ns + 1)])
                        bc = post_pools["b"].tile([128, SQH], FP, name="bc",
                                                  tag="bc")
                        nc.vector.reciprocal(bc[:], bc_ps[:])
                        nc.vector.tensor_tensor(norm_g[:, SQH * h:SQH * (h + 1)],
                                                outT_sb_box[0][:, h, :], bc[:],
                                                MULT)

                    if "F" not in phases:
                        return
                    # ---- output projection (4-quadrant: a2 on rows, h on cols) ----
                    fps = [ps_m.tile([128, D], FP, name="fp_a", tag="scores"),
                           ps_m.tile([128, D], FP, name="fp_b", tag="cpart",
                                     bufs=1)]
                    for a2 in range(2):
                        for ns in range(NSD):
                            nc.tensor.matmul(fps[a2][:, WD * ns:WD * (ns + 1)],
                                             ones16[0:1, 0:128],
                                             bo_sb[0:1, WD * ns:WD * (ns + 1)],
                                             start=True, stop=False,
                                             skip_group_check=True)
                    for ns in range(NSD):
                        for t in range(TT):
                            for a2 in range(2):
                                lo = 64 * a2
                                nc.tensor.matmul(
                                    fps[a2][:, WD * ns:WD * (ns + 1)],
                                    norm_g[lo:lo + 64, t::TT],
                                    woT_dup[lo:lo + 64, t, WD * ns:WD * (ns + 1)],
                                    tile_position=(lo, 0),
                                    start=False, stop=(t == TT - 1),
                                    skip_group_check=True)
                    for a2 in range(2):
                        fsb = post_pools["f"].tile([128, D], FP, name="fsb",
                                                   tag="fsb")
                        nc.vector.tensor_copy(fsb[:], fps[a2][:])
                        al = 2 * g + a2
                        nc.sync.dma_start(out_d[128 * al:128 * (al + 1), :],
                                          fsb[:])

                return nf

            pending_nf[0] = make_nf()

        if pending_nf[0] is not None:
            pending_nf[0]()
            pending_nf[0] = None
        if loop_cm is not None:
            loop_cm.__exit__(None, None, None)

    return nc


def _split_excess_waits(nc, max_waits=1):
    """This toolchain's walrus accepts only one sync-wait per instruction;
    hoist extra waits onto NoOps inserted just before."""
    fn = nc.m.functions[0]
    n_new = 0
    for blk in fn.blocks:
        new_insts = []
        for inst in blk.instructions:
            si = getattr(inst, 'sync_info', None)
            if si is not None and si.on_wait is not None \
                    and len(si.on_wait) > max_waits:
                waits = list(si.on_wait)
                while len(waits) > max_waits:
                    chunk, waits = waits[:max_waits], waits[max_waits:]
                    n_new += 1
                    new_insts.append(mybir.InstNoOp(
                        name=f"I-waitsplit-{n_new}", engine=inst.engine,
                        ins=[], outs=[],
                        sync_info=mybir.SyncInfo(on_wait=chunk, on_update=[]),
                        bass_nofuse=True))
                si.on_wait = waits
            new_insts.append(inst)
        blk.instructions = new_insts
    return n_new


class _PjrtRunner:
    def __init__(self, nc, n_cores):
        import jax
        from jax.sharding import Mesh, PartitionSpec
        from jax.experimental.shard_map import shard_map
        from concourse.bass2jax import (_bass_exec_p, partition_id_tensor,
                                        install_neuronx_cc_hook)
        install_neuronx_cc_hook()
        self.jax = jax
        self.n_cores = n_cores
        pname = nc.partition_id_tensor.name if nc.partition_id_tensor else None
        in_names, out_names, out_avals, zero_outs = [], [], [], []
        for alloc in nc.m.functions[0].allocations:
            if not isinstance(alloc, mybir.MemoryLocationSet):
                continue
            name = alloc.memorylocations[0].name
            if alloc.kind == "ExternalInput":
                if name != pname:
                    in_names.append(name)
            elif alloc.kind == "ExternalOutput":
                shape = tuple(alloc.tensor_shape)
                dtype = mybir.dt.np(alloc.dtype)
                out_names.append(name)
                out_avals.append(jax.core.ShapedArray(shape, dtype))
                zero_outs.append(np.zeros(shape, dtype))
        self.in_names, self.out_names = in_names, out_names
        self.out_avals, self.zero_outs = out_avals, zero_outs
        n_params, n_outs = len(in_names), len(out_avals)
        self.n_params = n_params
        all_in = in_names + out_names + ([pname] if pname else [])

        def _body(*args):
            operands = list(args)
            if pname is not None:
                operands.append(partition_id_tensor())
            return tuple(_bass_exec_p.bind(
                *operands, out_avals=tuple(out_avals), in_names=tuple(all_in),
                out_names=tuple(out_names), lowering_input_output_aliases=(),
                sim_require_finite=True, sim_require_nnan=True, nc=nc))

        devices = jax.devices()[:n_cores]
        self.mesh = Mesh(np.asarray(devices), ("core",))
        in_specs = (PartitionSpec("core"),) * (n_params + n_outs)
        out_specs = (PartitionSpec("core"),) * n_outs
        self.fn = jax.jit(
            shard_map(_body, mesh=self.mesh, in_specs=in_specs,
                      out_specs=out_specs, check_rep=False), keep_unused=True)
        self.PartitionSpec = PartitionSpec

    def run(self, in_maps):
        jax = self.jax
        per_core = [[np.asarray(m[n]) for n in self.in_names] for m in in_maps]
        concat_in = [np.concatenate([per_core[c][i] for c in range(self.n_cores)],
                                    axis=0) for i in range(self.n_params)]
        concat_zeros = [np.zeros((self.n_cores * z.shape[0], *z.shape[1:]),
                                 z.dtype) for z in self.zero_outs]
        sharding = jax.sharding.NamedSharding(self.mesh, self.PartitionSpec("core"))
        dev_in = [jax.device_put(a, sharding) for a in concat_in + concat_zeros]
        outs = self.fn(*dev_in)
        jax.block_until_ready(outs)
        return [
            {n: np.asarray(outs[i]).reshape(self.n_cores,
                                            *self.out_avals[i].shape)[c]
             for i, n in enumerate(self.out_names)}
            for c in range(self.n_cores)
        ]


_CACHE = {}

B_, S_, D_, H_, DK_ = 4, 2048, 1024, 16, 64
HL_ = H_ // 2          # heads per device
EL_ = HL_ * DK_        # value-projection width per device

# bc broadcast selector: rows {0: csum a2=0, 1: diag a2=0, 2: diag a2=1,
# 32: csum a2=1}; a2=0 -> partitions 0-63, a2=1 -> partitions 64-127
_SEL = np.zeros((33, 128), np.float32)
_SEL[0:2, 0:64] = 1.0
_SEL[2:3, 64:128] = 1.0
_SEL[32:33, 64:128] = 1.0


def _make_in_maps(x, Wv, bv, Wo, bo):
    woT = np.ascontiguousarray(Wo.T).astype(np.float16)
    bo16 = bo.reshape(1, -1).astype(np.float16)
    maps = []
    for dev in range(8):
        b, hg = dev // 2, dev % 2
        maps.append({
            "xT": np.ascontiguousarray(x[b].T).astype(np.float16),
            "wvT": np.ascontiguousarray(
                Wv[EL_ * hg:EL_ * (hg + 1), :].T).astype(np.float16),
            "woT": woT,
            "bv": bv[EL_ * hg:EL_ * (hg + 1)].reshape(1, -1).astype(np.float16),
            "bo": bo16,
            "sel": _SEL,
        })
    return maps


def kernel(x, Wv, bv, Wo, bo):
    x, Wv, bv = np.asarray(x), np.asarray(Wv), np.asarray(bv)
    Wo, bo = np.asarray(Wo), np.asarray(bo)
    if "r" not in _CACHE:
        nc = _build_mha_nc(S=S_, D=D_, HL=HL_, dk=DK_)
        _split_excess_waits(nc)
        _CACHE["r"] = _PjrtRunner(nc, 8)
    r = _CACHE["r"]
    res = r.run(_make_in_maps(x, Wv, bv, Wo, bo))
    out = np.zeros((B_, S_, D_), np.float32)
    for dev in range(8):
        b, hg = dev // 2, dev % 2
        out[b, 1024 * hg:1024 * (hg + 1), :] = res[dev]["out"]
    return out


# revision 17
# speedup vs baseline: 24.7620x; 1.2969x over previous
"""Trainium2 Bass kernel for nn_MultiHeadAttention_910533067646.

Self-contained: builds the Bass module, shards the full inputs across the
8 NeuronCores (data-parallel over batch x tensor-parallel over heads), runs
via PJRT, and reassembles the full output.

The reference module applies one shared projection p = x @ Wv.T + bv for
q=k=v, per-head softmax(p ph.T/8) @ ph, then a head-major (bugged) reshape
and output projection. The bugged reshape maps each head's attention output
to a disjoint 128-row block of the final output, so no cross-device
reduction is needed: device (b, hg) computes output rows
[1024*hg, 1024*hg+1024) of batch b.

v2 engine split: the softmax exp over S x S scores is the elementwise
bottleneck, so it is spread across three engines. Scores are symmetric
(q=k), so for each 128-row q-block only one of the two 1024-col halves
contains the diagonal block. Diagonal-half tiles (plus one fixed off-diag
row per half) run exact exp+row-accumulate on the Activation engine; the
remaining off-diagonal tiles are computed by a Schraudolph bit-trick exp on
the Vector engine (y = int32(s*a+b) bitcast as fp32) and their softmax-
denominator contributions come from GpSimd partition-axis column sums
(valid by symmetry). The per-column denominators are assembled k-indexed by
a ones-matmul broadcast, reciprocated on DVE, and applied to the
attention-value product.
"""
import math
import numpy as np

from collections import deque
from contextlib import ExitStack

import concourse.bass as bass
import concourse.mybir as mybir
import concourse.tile as tile
from concourse.masks import make_identity

FP = mybir.dt.float32
FP16 = mybir.dt.float16
BF16 = mybir.dt.bfloat16
I16 = mybir.dt.int16
Exp = mybir.ActivationFunctionType.Exp
ADD = mybir.AluOpType.add
MULT = mybir.AluOpType.mult

BIAS = -15.0                                  # static logit shift (softmax-invariant)
SCHR_SCALE = 2.0 ** 7 / math.log(2.0)         # 184.664965 (bf16 target)
SCHR_A = SCHR_SCALE / 8.0                     # exp arg is scores/8
SCHR_B = 127.0 * 2.0 ** 7 - 7.35 + BIAS * SCHR_SCALE


def _build_mha_nc(S=2048, D=1024, HL=8, dk=64, phases="ABCNF",
                  loop_bcnf=1, dbg=False):
    EL = HL * dk            # local width of the value projection
    KK = D // 128           # contraction k-tiles
    NG = HL // 2            # head pairs
    NB = S // 128           # 128-row blocks of the sequence
    NBH = NB // 2           # blocks per sq-half
    SQH = S // 2            # sq-half width
    TT = D // dk            # total heads (= reshape block count)
    W = min(512, SQH)       # N-slice width for panels
    NSL = SQH // W
    WS = min(512, S)        # N-slice for pT phase
    NSS = S // WS
    WD = min(512, D)        # N-slice over D (output projection)
    NSD = D // WD
    ACT_ROW = NBH - 1       # off-diag row (i % NBH) handled by Act engine
    POOLW = ACT_ROW * 128   # gpsimd col-reduce width (excludes ACT_ROW block)
    assert EL <= 512 and SQH == D and S == 128 * TT and TT % 2 == 0

    nc = bass.Bass("TRN2")
    xT_d = nc.dram_tensor("xT", [D, S], FP16, kind="ExternalInput")
    wvT_d = nc.dram_tensor("wvT", [D, EL], FP16, kind="ExternalInput")
    woT_d = nc.dram_tensor("woT", [D, D], FP16, kind="ExternalInput")
    bv_d = nc.dram_tensor("bv", [1, EL], FP16, kind="ExternalInput")
    bo_d = nc.dram_tensor("bo", [1, D], FP16, kind="ExternalInput")
    sel_d = nc.dram_tensor("sel", [33, 128], FP, kind="ExternalInput")
    out_d = nc.dram_tensor("out", [128 * HL, D], FP, kind="ExternalOutput")

    with ExitStack() as stk:
        tc = stk.enter_context(tile.TileContext(nc))
        const = stk.enter_context(tc.tile_pool(name="const", bufs=1))
        ppool = stk.enter_context(tc.tile_pool(name="ppool", bufs=1))
        epool = stk.enter_context(tc.tile_pool(name="epool", bufs=8))
        rpool = stk.enter_context(tc.tile_pool(name="rpool", bufs=4))
        ps_m = stk.enter_context(tc.tile_pool(name="ps_m", bufs=2, space="PSUM"))

        bv_sb = const.tile([1, EL], FP16, name="bv_sb")
        bo_sb = const.tile([1, D], FP16, name="bo_sb")
        ones16 = const.tile([1, 512], FP16, name="ones16")
        sel_sb = const.tile([33, 128], FP, name="sel_sb")
        ident = const.tile([128, 128], FP, name="ident")
        identB = const.tile([128, 128], BF16, name="identB")
        bias_sb = const.tile([128, 1], FP, name="bias_sb")
        nc.sync.dma_start(bv_sb[:], bv_d[:])
        nc.sync.dma_start(bo_sb[:], bo_d[:])
        ones_bf = const.tile([128, 1], BF16, name="ones_bf")
        nc.gpsimd.memset(ones16[:], 1.0)
        nc.gpsimd.memset(ones_bf[:], 1.0)
        nc.gpsimd.memset(bias_sb[:], BIAS)
        nc.sync.dma_start(sel_sb[:], sel_d[:])
        make_identity(nc, ident[:])
        make_identity(nc, identB[:])

        pT_sb = ppool.tile([128, NG, S], BF16, name="pT_sb")
        p_sb = ppool.tile([128, NB, EL], BF16, name="p_sb")

        xt_ctx = tc.tile_pool(name="xtpool", bufs=1)
        xtpool = xt_ctx.__enter__()
        wvT_sb = xtpool.tile([128, KK, EL], FP16, name="wvT_sb")
        xT_sb = xtpool.tile([128, KK, S], FP16, name="xT_sb")
        nc.sync.dma_start(wvT_sb[:],
                          wvT_d[:].rearrange("(kk p) e -> p kk e", p=128))
        for kk in range(KK):
            nc.sync.dma_start(xT_sb[:, kk, :], xT_d[128 * kk:128 * (kk + 1), :])

        # ---- projection work units (phase A), emitted interleaved ----
        def emit_pT0(ns):
            ps = ps_m.tile([128, WS], FP, name="ps_pt", tag="scores")
            for kk in range(KK):
                nc.tensor.matmul(ps[:], wvT_sb[:, kk, 0:128],
                                 xT_sb[:, kk, WS * ns:WS * (ns + 1)],
                                 start=(kk == 0), stop=False)
            nc.tensor.matmul(ps[:], bv_sb[0:1, 0:128],
                             ones16[0:1, 0:WS], start=False, stop=True)
            nc.vector.tensor_copy(pT_sb[:, 0, WS * ns:WS * (ns + 1)], ps[:])

        def emit_p(j):
            ps = ps_m.tile([128, EL], FP, name="ps_p", tag="scores")
            for kk in range(KK):
                nc.tensor.matmul(ps[:], xT_sb[:, kk, 128 * j:128 * (j + 1)],
                                 wvT_sb[:, kk, :], start=(kk == 0), stop=False)
            nc.tensor.matmul(ps[:], ones16[0:1, 0:128], bv_sb[0:1, :],
                             start=False, stop=True)
            nc.vector.tensor_copy(p_sb[:, j, :], ps[:])

        def emit_T(g, j):
            ps = ps_m.tile([128, 128], BF16, name="ps_t0", tag="scores")
            nc.tensor.transpose(ps[:], p_sb[:, j, 128 * g:128 * (g + 1)],
                                identB[:])
            nc.vector.tensor_copy(pT_sb[:, g, 128 * j:128 * (j + 1)], ps[:])

        proj_q = deque()
        for r in range(NBH):
            proj_q.append(("p", r))
            proj_q.append(("p", NBH + r))
        for g in range(1, NG):
            for j in range(NB):
                proj_q.append(("T", g, j))
        p_left = [NB]

        post_pools = {}

        def ensure_post_pools():
            # opened once the p-projection is done: reuses xT address space
            if post_pools:
                return
            xt_ctx.__exit__(None, None, None)
            post_pools["w"] = stk.enter_context(tc.tile_pool(name="wpool", bufs=1))
            post_pools["n"] = stk.enter_context(tc.tile_pool(name="npool", bufs=2))
            post_pools["b"] = stk.enter_context(tc.tile_pool(name="bpool", bufs=2))
            post_pools["f"] = stk.enter_context(tc.tile_pool(name="fpool", bufs=2))
            woT_dup = post_pools["w"].tile([128, TT, D], FP16, name="woT_dup")
            src = woT_d[:].rearrange("(t p) e -> p t e", p=dk)
            nc.sync.dma_start(woT_dup[0:dk, :, :], src)
            nc.sync.dma_start(woT_dup[dk:2 * dk, :, :], src)
            post_pools["woT"] = woT_dup

        def emit_proj(n):
            while n > 0 and proj_q:
                u = proj_q.popleft()
                if u[0] == "p":
                    emit_p(u[1])
                    p_left[0] -= 1
                    if p_left[0] == 0:
                        ensure_post_pools()
                else:
                    emit_T(u[1], u[2])
                n -= 1

        for ns in range(NSS):
            emit_pT0(ns)

        if "B" not in phases:
            emit_proj(len(proj_q))
            ensure_post_pools()

        loop_cm = None
        if loop_bcnf > 1:
            emit_proj(len(proj_q))
            ensure_post_pools()
            loop_cm = tc.For_i(0, loop_bcnf, 1)
            loop_cm.__enter__()
        pending_nf = deque()
        for g in range(NG if "B" in phases else 0):
            sums = epool.tile([128, 2, NB, 2], FP, name="sums", tag="sums", bufs=2)
            nc.vector.memset(sums[:], 0.0)
            outT_sb_box = [None]
            cpart = [None, None]
            rows_hh = [None, None]

            W16 = min(512, SQH)
            NS16 = SQH // W16

            csum_box = [None]

            def emit_csum(h, i, a2, E):
                # colsum of off-diag tile accumulated over the 8 tiles of
                # this (h, a2) group (PE ones-matmul; excludes ACT_ROW block;
                # the two a2 groups run in different PE column groups)
                i0 = NBH * (1 - h)
                if csum_box[0] is None:
                    csum_box[0] = ps_m.tile([33, SQH], FP, name="csum",
                                            tag="csum", bufs=1)
                for ns in range(NSL):
                    wlo, whi = W * ns, min(W * (ns + 1), POOLW)
                    if wlo >= POOLW:
                        break
                    nc.tensor.matmul(csum_box[0][32 * a2:32 * a2 + 1, wlo:whi],
                                     ones_bf[:], E[:, wlo:whi],
                                     tile_position=(0, 32 * a2),
                                     start=(i == i0), stop=(i == i0 + NBH - 1),
                                     skip_group_check=True)

            def emit_C_one(h, i, ns, a2, E, i_first, i_last):
                al = 2 * g + a2
                if cpart[h] is None:
                    cpart[h] = ps_m.tile([128, SQH], FP, name="cp",
                                         tag="cpart", bufs=1)
                nc.tensor.matmul(
                    cpart[h][64 * a2:64 * (a2 + 1), W16 * ns:W16 * (ns + 1)],
                    p_sb[:, i, dk * al:dk * (al + 1)],
                    E[:, W16 * ns:W16 * (ns + 1)],
                    tile_position=(0, 64 * a2),
                    start=(i == i_first), stop=(i == i_last),
                    skip_group_check=True)

            def drain_C(h):
                if outT_sb_box[0] is None:
                    outT_sb_box[0] = post_pools["n"].tile(
                        [128, 2, SQH], FP, name="outT_sb", tag="outT_sb", bufs=2)
                nc.vector.tensor_copy(outT_sb_box[0][:, h, :], cpart[h][:])
                cpart[h] = None

            # N/F stages of the previous pair run at these (h, step) slots
            NF_SLOTS = {(0, 1), (0, 5), (0, 9), (0, 13), (1, 9), (1, 13)}

            for h in range(2):
                rows_h = rpool.tile([33, SQH], FP, name="rows", tag="rows", bufs=4)
                rows_hh[h] = rows_h
                nc.gpsimd.memset(rows_h[:], 0.0)
                # interleave diag (Act) and off-diag (DVE) steps so both
                # engines stay busy concurrently instead of convoying
                d0, o0 = NBH * h, NBH * (1 - h)
                order = []
                for r in range(NBH):
                    order.append(d0 + r)
                    order.append(o0 + r)
                i_first, i_last = order[0], order[-1]
                prev = None
                for step, i in enumerate(order):
                    emit_proj(2)
                    if pending_nf and (h, step) in NF_SLOTS:
                        pending_nf.popleft()()
                    cur = []
                    for a2 in range(2):
                        lo, hi = 64 * a2, 64 * (a2 + 1)
                        # C matmuls of the previous step first: they depend
                        # only on E(prev), so they fill the PE gap while the
                        # current exp still runs
                        if prev is not None and "C" in phases:
                            pi = prev[0]
                            for k in range(NS16):
                                ns = (k + a2) % NS16
                                emit_C_one(h, pi, ns, a2, prev[1][a2],
                                           i_first, i_last)
                            if (pi // NBH) != h:
                                emit_csum(h, pi, a2, prev[1][a2])
                        sc = ps_m.tile([128, SQH], FP, name="sc", tag="scores")
                        for ns in range(NSL):
                            nc.tensor.matmul(
                                sc[:, W * ns:W * (ns + 1)],
                                pT_sb[lo:hi, g, 128 * i:128 * (i + 1)],
                                pT_sb[lo:hi, g,
                                      SQH * h + W * ns:SQH * h + W * (ns + 1)],
                                tile_position=(64 * a2, 0))
                        E = epool.tile([128, SQH], BF16, name="E", tag="E", bufs=8)
                        diag = (i // NBH) == h
                        if diag or (i % NBH) == ACT_ROW:
                            nc.scalar.activation(
                                E[:], sc[:], Exp,
                                scale=0.125, bias=bias_sb[:, 0:1],
                                accum_out=sums[:, a2, i, h:h + 1])
                        else:
                            nc.vector.tensor_scalar(
                                E[:].bitcast(I16), sc[:],
                                SCHR_A, SCHR_B, MULT, ADD)
                        cur.append(E)
                    prev = (i, cur)
                if "C" in phases:
                    pi = prev[0]
                    for k in range(NS16):
                        for a2 in range(2):
                            emit_C_one(h, pi, (k + a2) % NS16, a2, prev[1][a2],
                                       i_first, i_last)
                    for a2 in range(2):
                        if (pi // NBH) != h:
                            emit_csum(h, pi, a2, prev[1][a2])
                    drain_C(h)
                    # csum rows -> rows_h (lane-aligned copies: partitions 0, 32)
                    nc.vector.tensor_copy(rows_h[0:1, 0:POOLW],
                                          csum_box[0][0:1, 0:POOLW])
                    nc.scalar.copy(rows_h[32:33, 0:POOLW],
                                   csum_box[0][32:33, 0:POOLW])
                    csum_box[0] = None

            emit_proj(len(proj_q))  # flush any phase-A leftovers
            ensure_post_pools()
            woT_dup = post_pools["woT"]
            if "N" not in phases:
                continue

            # ---- normalization + output projection, staged for overlap ----

            def make_nf_stages(g=g, sums=sums, outT_sb_box=outT_sb_box,
                               rows_hh=tuple(rows_hh)):
                box = {}

                def s1_tot():
                    # diag-half row sums, transposed to k-indexing
                    totT = rpool.tile([NB, 2, 128], FP, name="totT",
                                      tag="totT", bufs=2)
                    tot = epool.tile([128, 2, NB], FP, name="tot", tag="tot",
                                     bufs=2)
                    for a2 in range(2):
                        nc.vector.tensor_tensor(tot[:, a2, :], sums[:, a2, :, 0],
                                                sums[:, a2, :, 1], ADD)
                        ps_t = ps_m.tile([NB, 128], FP, name="ps_tt", tag="scores")
                        nc.tensor.transpose(ps_t[:], tot[:, a2, :], ident[:])
                        nc.vector.tensor_copy(totT[:, a2, :], ps_t[:])
                    for h in range(2):
                        for a2 in range(2):
                            nc.sync.dma_start(
                                rows_hh[h][a2 + 1:a2 + 2, :],
                                totT[NBH * h:NBH * (h + 1), a2, :])
                    box["norm"] = post_pools["n"].tile([128, S], FP16,
                                                       name="norm_g", tag="nr")

                def mk_s2(h):
                    def s2_bc():
                        rows_h = rows_hh[h]
                        # bc[p, n] = D[n] for the a2-half p belongs to
                        bc_ps = ps_m.tile([128, SQH], FP, name="bc_ps",
                                          tag="scores")
                        for ns in range(NSL):
                            nc.tensor.matmul(bc_ps[:, W * ns:W * (ns + 1)],
                                             sel_sb[:],
                                             rows_h[:, W * ns:W * (ns + 1)])
                        bc = post_pools["b"].tile([128, SQH], FP, name="bc",
                                                  tag="bc")
                        nc.vector.reciprocal(bc[:], bc_ps[:])
                        nc.vector.tensor_tensor(
                            box["norm"][:, SQH * h:SQH * (h + 1)],
                            outT_sb_box[0][:, h, :], bc[:], MULT)
                    return s2_bc

                def mk_s3(ns):
                    def s3_f():
                        norm_g = box["norm"]
                        if ns == 0:
                            box["fps"] = [
                                ps_m.tile([128, D], FP, name="fp_a",
                                          tag="scores"),
                                ps_m.tile([128, D], FP, name="fp_b",
                                          tag="cpart", bufs=1)]
                            for a2 in range(2):
                                for ns2 in range(NSD):
                                    nc.tensor.matmul(
                                        box["fps"][a2][:, WD * ns2:WD * (ns2 + 1)],
                                        ones16[0:1, 0:128],
                                        bo_sb[0:1, WD * ns2:WD * (ns2 + 1)],
                                        start=True, stop=False,
                                        skip_group_check=True)
                        for t in range(TT):
                            for a2 in range(2):
                                lo = 64 * a2
                                nc.tensor.matmul(
                                    box["fps"][a2][:, WD * ns:WD * (ns + 1)],
                                    norm_g[lo:lo + 64, t::TT],
                                    woT_dup[lo:lo + 64, t, WD * ns:WD * (ns + 1)],
                                    tile_position=(lo, 0),
                                    start=False, stop=(t == TT - 1),
                                    skip_group_check=True)
                        if ns == NSD - 1:
                            for a2 in range(2):
                                fsb = post_pools["f"].tile([128, D], FP,
                                                           name="fsb", tag="fsb")
                                nc.vector.tensor_copy(fsb[:], box["fps"][a2][:])
                                al = 2 * g + a2
                                nc.sync.dma_start(
                                    out_d[128 * al:128 * (al + 1), :], fsb[:])
                    return s3_f

                stages = [s1_tot, mk_s2(0), mk_s2(1)]
                if "F" in phases:
                    stages += [mk_s3(ns) for ns in range(NSD)]
                return stages

            pending_nf.extend(make_nf_stages())

        while pending_nf:
            pending_nf.popleft()()
        if loop_cm is not None:
            loop_cm.__exit__(None, None, None)

    return nc


def _split_excess_waits(nc, max_waits=1):
    """This toolchain's walrus accepts only one sync-wait per instruction;
    hoist extra waits onto NoOps inserted just before."""
    fn = nc.m.functions[0]
    n_new = 0
    for blk in fn.blocks:
        new_insts = []
        for inst in blk.instructions:
            si = getattr(inst, 'sync_info', None)
            if si is not None and si.on_wait is not None \
                    and len(si.on_wait) > max_waits:
                waits = list(si.on_wait)
                while len(waits) > max_waits:
                    chunk, waits = waits[:max_waits], waits[max_waits:]
                    n_new += 1
                    new_insts.append(mybir.InstNoOp(
                        name=f"I-waitsplit-{n_new}", engine=inst.engine,
                        ins=[], outs=[],
                        sync_info=mybir.SyncInfo(on_wait=chunk, on_update=[]),
                        bass_nofuse=True))
                si.on_wait = waits
            new_insts.append(inst)
        blk.instructions = new_insts
    return n_new


class _PjrtRunner:
    def __init__(self, nc, n_cores):
        import jax
        from jax.sharding import Mesh, PartitionSpec
        from jax.experimental.shard_map import shard_map
        from concourse.bass2jax import (_bass_exec_p, partition_id_tensor,
                                        install_neuronx_cc_hook)
        install_neuronx_cc_hook()
        self.jax = jax
        self.n_cores = n_cores
        pname = nc.partition_id_tensor.name if nc.partition_id_tensor else None
        in_names, out_names, out_avals, zero_outs = [], [], [], []
        for alloc in nc.m.functions[0].allocations:
            if not isinstance(alloc, mybir.MemoryLocationSet):
                continue
            name = alloc.memorylocations[0].name
            if alloc.kind == "ExternalInput":
                if name != pname:
                    in_names.append(name)
            elif alloc.kind == "ExternalOutput":
                shape = tuple(alloc.tensor_shape)
                dtype = mybir.dt.np(alloc.dtype)
                out_names.append(name)
                out_avals.append(jax.core.ShapedArray(shape, dtype))
                zero_outs.append(np.zeros(shape, dtype))
        self.in_names, self.out_names = in_names, out_names
        self.out_avals, self.zero_outs = out_avals, zero_outs
        n_params, n_outs = len(in_names), len(out_avals)
        self.n_params = n_params
        all_in = in_names + out_names + ([pname] if pname else [])

        def _body(*args):
            operands = list(args)
            if pname is not None:
                operands.append(partition_id_tensor())
            return tuple(_bass_exec_p.bind(
                *operands, out_avals=tuple(out_avals), in_names=tuple(all_in),
                out_names=tuple(out_names), lowering_input_output_aliases=(),
                sim_require_finite=True, sim_require_nnan=True, nc=nc))

        devices = jax.devices()[:n_cores]
        self.mesh = Mesh(np.asarray(devices), ("core",))
        in_specs = (PartitionSpec("core"),) * (n_params + n_outs)
        out_specs = (PartitionSpec("core"),) * n_outs
        self.fn = jax.jit(
            shard_map(_body, mesh=self.mesh, in_specs=in_specs,
                      out_specs=out_specs, check_rep=False), keep_unused=True)
        self.PartitionSpec = PartitionSpec

    def run(self, in_maps):
        jax = self.jax
        per_core = [[np.asarray(m[n]) for n in self.in_names] for m in in_maps]
        concat_in = [np.concatenate([per_core[c][i] for c in range(self.n_cores)],
                                    axis=0) for i in range(self.n_params)]
        concat_zeros = [np.zeros((self.n_cores * z.shape[0], *z.shape[1:]),
                                 z.dtype) for z in self.zero_outs]
        sharding = jax.sharding.NamedSharding(self.mesh, self.PartitionSpec("core"))
        dev_in = [jax.device_put(a, sharding) for a in concat_in + concat_zeros]
        outs = self.fn(*dev_in)
        jax.block_until_ready(outs)
        return [
            {n: np.asarray(outs[i]).reshape(self.n_cores,
                                            *self.out_avals[i].shape)[c]
             for i, n in enumerate(self.out_names)}
            for c in range(self.n_cores)
        ]


_CACHE = {}

B_, S_, D_, H_, DK_ = 4, 2048, 1024, 16, 64
HL_ = H_ // 2          # heads per device
EL_ = HL_ * DK_        # value-projection width per device

# bc broadcast selector: rows {0: csum a2=0, 1: diag a2=0, 2: diag a2=1,
# 32: csum a2=1}; a2=0 -> partitions 0-63, a2=1 -> partitions 64-127
_SEL = np.zeros((33, 128), np.float32)
_SEL[0:2, 0:64] = 1.0
_SEL[2:3, 64:128] = 1.0
_SEL[32:33, 64:128] = 1.0


def _make_in_maps(x, Wv, bv, Wo, bo):
    woT = np.ascontiguousarray(Wo.T).astype(np.float16)
    bo16 = bo.reshape(1, -1).astype(np.float16)
    maps = []
    for dev in range(8):
        b, hg = dev // 2, dev % 2
        maps.append({
            "xT": np.ascontiguousarray(x[b].T).astype(np.float16),
            "wvT": np.ascontiguousarray(
                Wv[EL_ * hg:EL_ * (hg + 1), :].T).astype(np.float16),
            "woT": woT,
            "bv": bv[EL_ * hg:EL_ * (hg + 1)].reshape(1, -1).astype(np.float16),
            "bo": bo16,
            "sel": _SEL,
        })
    return maps


def kernel(x, Wv, bv, Wo, bo):
    x, Wv, bv = np.asarray(x), np.asarray(Wv), np.asarray(bv)
    Wo, bo = np.asarray(Wo), np.asarray(bo)
    if "r" not in _CACHE:
        nc = _build_mha_nc(S=S_, D=D_, HL=HL_, dk=DK_)
        _split_excess_waits(nc)
        _CACHE["r"] = _PjrtRunner(nc, 8)
    r = _CACHE["r"]
    res = r.run(_make_in_maps(x, Wv, bv, Wo, bo))
    out = np.zeros((B_, S_, D_), np.float32)
    for dev in range(8):
        b, hg = dev // 2, dev % 2
        out[b, 1024 * hg:1024 * (hg + 1), :] = res[dev]["out"]
    return out
